# revision 38
# baseline (speedup 1.0000x reference)
"""2-layer GCN on 8 Trainium2 NeuronCores (Bass/Tile).

Math: gcn_conv(x, W, b) = D^-1/2 (A+I) D^-1/2 (x W) + b.  With
x~ = dinv*x and g~ = dinv*(h W2)  (both tables dinv-row-scaled):
  h[v]   = relu((sum_{u->v} dinv_v x~[u] + dinv_v x~[v]) W1 + b1)
  out[v] = sum_{u->v} dinv_v g~[u] + dinv_v g~[v] + b2
Self-loops ride the edge streams as ordinary edges; dinv_v lives in the
one-hot values, dinv_u in the gathered tables.

Distribution: dst nodes sharded 8 ways (12500/core, identity layout).
g~ is stored 128-wide (64 real cols) so rows are 256B for dma_gather;
one 8-rank AllGather (25.6MB) assembles g_full between the layers.

ONE shared edge schedule serves both layers (same src row index per
edge, tables have identical [N,128] shape): per-core slot order, idx
(int16 into 4 row-blocks of 25000) and meta ([dstl, dinv_dst] bf16) are
built once on the host and PRELOADED to SBUF (no per-call idx/meta
streams).  Consumption is tile-major: per mm entry a one-hot
(is_equal x dinv_dst, bf16) routes 128 gathered rows to their dst
columns via a PE matmul accumulating in PSUM.
- L1: stationary = msgs, moving = one-hot -> pt [feat, dst];
  epilogue: W1 -> relu -> W2 -> g~ rows (ACT per-partition dinv scale).
- L2: FLIPPED: stationary = one-hot, moving = msgs[:, :64] ->
  pt [dst, 64]; epilogue: +b2 (one DVE add per 4 tiles), staged output
  flushes of 1KB/partition.
- One-hot builds alternate DVE / Pool (GPSIMD) so neither sequencer
  bottlenecks behind the gather DMA stream.
"""
import sys

sys.path.insert(0, "/opt/trn_rl_repo")
import numpy as np
import ml_dtypes

BF16 = ml_dtypes.bfloat16
NC = 8
CIN, CH, COUT = 128, 128, 64
CALL_CHUNKS = 40  # gather-call granularity (chunks of 128 descs)
POOL_MOD = 3  # mm entry i builds on Pool when i % POOL_MOD == 2
WBW = 752  # wb cols: W1 0:128 | W2 128:192 | iota 256:384 | b1 384 |
#            b2bc4 486:742


def _balance(nv, caps, k_tb):
    """Best-fit-decreasing node->tile assignment for one core.

    nv: [NSH, NBLK] per-node block in-degree vectors.
    caps: [TILES] node slots per tile.  k_tb: [TILES, NBLK] edge caps.
    Returns pos[NSH] (node -> slot = tile*128 + slot_in_tile) or None.
    """
    NSH, NBLK = nv.shape
    TILES = caps.shape[0]
    capv = k_tb.astype(np.float64)
    np.maximum(capv, 1e-9, out=capv)
    usedv = np.zeros((TILES, NBLK), dtype=np.float64)
    usect = np.zeros(TILES, dtype=np.int64)
    pos = np.empty(NSH, dtype=np.int64)
    order = np.argsort(-nv.sum(1), kind="stable")
    for n in order:
        v = nv[n].astype(np.float64)
        post = (usedv + v[None, :]) / capv
        feas = (post <= 1.0).all(1) & (usect < caps)
        if not feas.any():
            return None
        score = np.maximum(post.max(1), (usect + 1) / caps)
        score[~feas] = np.inf
        t = int(np.argmin(score))
        usedv[t] += v
        pos[n] = t * 128 + usect[t]
        usect[t] += 1
    return pos


def _schedule(edge_index, n_nodes):
    N = n_nodes
    NSH = N // NC
    TILES = (NSH + 127) // 128
    assert N % NC == 0

    src = np.asarray(edge_index[0], dtype=np.int64)
    dst = np.asarray(edge_index[1], dtype=np.int64)
    E = src.shape[0]
    deg = np.bincount(dst, minlength=N).astype(np.float64) + 1.0
    dinv = 1.0 / np.sqrt(deg)

    s = src
    d = dst
    EA = s.shape[0]

    core = d // NSH
    j = d - core * NSH
    norm = dinv[d].astype(np.float32)

    NBLK = 4
    BLK = (N + NBLK - 1) // NBLK
    assert BLK <= 32767 and (NSH % BLK == 0 or BLK % NSH == 0)
    m1 = s % BLK
    blk = s // BLK  # src block 0..3; self-loops get dedicated block below
    ncell = NBLK * TILES

    # node -> tile permutation balancing per-(tile, block) cell loads
    nv = np.bincount(
        (core * NSH + j) * NBLK + blk, minlength=NC * NSH * NBLK
    ).reshape(NC, NSH, NBLK)
    B = nv.sum(axis=1)  # [NC, NBLK]
    caps_n = np.full(TILES, 128, dtype=np.int64)
    caps_n[TILES - 1] = NSH - 128 * (TILES - 1)
    slack = 512
    for _attempt in range(8):
        TOTb = ((B.max(axis=0) + slack + 127) // 128) * 128
        captb = np.zeros((NBLK, TILES), dtype=np.int64)
        for b in range(NBLK):
            tgt = TOTb[b] * caps_n / NSH
            base = np.floor(tgt).astype(np.int64)
            rem = int(TOTb[b] - base.sum())
            order_t = np.argsort(-(tgt - base))
            base[order_t[:rem]] += 1
            captb[b] = base
        pos_all = np.empty((NC, NSH), dtype=np.int64)
        ok = True
        for r in range(NC):
            pos = _balance(nv[r], caps_n, captb.T)
            if pos is None:
                ok = False
                break
            pos_all[r] = pos
        if ok:
            break
        slack *= 2
    assert ok, "balance failed"

    pos_d = pos_all[core, j]
    t_dst = pos_d >> 7
    dstl = (pos_d & 127).astype(np.float32)
    # block 0 = SELF (tile-aligned, 128/tile); blocks 1..4 = src ranges
    cap_self = np.full((1, TILES), 128, dtype=np.int64)
    captb5 = np.concatenate([cap_self, captb], axis=0)  # [NBLK+1, TILES]
    NBLK1 = NBLK + 1
    ncell1 = NBLK1 * TILES
    cell = (blk + 1) * TILES + t_dst
    caps = captb5.reshape(-1)  # [ncell1], block-major

    off = np.zeros(ncell1 + 1, dtype=np.int64)
    np.cumsum(caps, out=off[1:])
    CAP = int(off[-1])
    assert CAP % 128 == 0
    C = CAP // 128

    # slot: per (core, cell), streamed edges laid in (m1) order
    key = (core * ncell1 + cell) * BLK + m1
    order = np.argsort(key, kind="stable")
    ckey = (core * ncell1 + cell)[order]
    cell_first = np.zeros(EA, dtype=bool)
    cell_first[0] = True
    np.not_equal(ckey[1:], ckey[:-1], out=cell_first[1:])
    cstart = np.nonzero(cell_first)[0]
    c_id = np.cumsum(cell_first) - 1
    pos_in_cell = np.arange(EA, dtype=np.int64) - cstart[c_id]
    assert (pos_in_cell < caps[ckey % ncell1]).all()
    slot_sorted = off[ckey % ncell1] + pos_in_cell
    slot = np.empty(EA, dtype=np.int64)
    slot[order] = slot_sorted

    # L2 source row: permuted shard-major g_full row, same block as src
    sc = s // NSH
    sj = s - sc * NSH
    g_row = sc * NSH + pos_all[sc, sj]
    m2 = g_row - blk * BLK
    assert (m2 >= 0).all() and (m2 < BLK).all()

    idx1_all = np.zeros((NC, CAP), dtype=np.int16)
    idx2_all = np.zeros((NC, CAP), dtype=np.int16)
    dstl_a = np.full((NC, CAP), -1.0, dtype=np.float32)
    norm_a = np.zeros((NC, CAP), dtype=np.float32)
    idx1_all[core, slot] = m1.astype(np.int16)
    idx2_all[core, slot] = m2.astype(np.int16)
    dstl_a[core, slot] = dstl
    norm_a[core, slot] = norm
    # self block: node with permuted position p sits at slot p; idx1 = its
    # original j (row of x_own), idx2 = p (row of g_shard), value = dinv
    for r in range(NC):
        jj = np.arange(NSH, dtype=np.int64)
        p = pos_all[r]
        idx1_all[r, p] = jj.astype(np.int16)
        idx2_all[r, p] = p.astype(np.int16)
        dstl_a[r, p] = (p & 127).astype(np.float32)
        norm_a[r, p] = dinv[r * NSH + jj].astype(np.float32)

    # mm entries, tile-major consumption order
    mm = []
    for t in range(TILES):
        for b in range(NBLK1):
            o = int(off[b * TILES + t])
            cap = int(caps[b * TILES + t])
            if cap == 0:
                continue
            c0, c1 = o >> 7, (o + cap - 1) >> 7
            for c in range(c0, c1 + 1):
                lo = max(o, c * 128)
                hi = min(o + cap, (c + 1) * 128)
                mm.append((b, t, c, lo, hi))
    mm = np.array(mm, dtype=np.int64)
    M = mm.shape[0]

    md = np.full((NC, M, 128), -1.0, dtype=np.float32)
    mn = np.zeros((NC, M, 128), dtype=np.float32)
    for i in range(M):
        b, t, c, lo, hi = mm[i]
        base = int(c) * 128
        md[:, i, lo - base : hi - base] = dstl_a[:, lo:hi]
        mn[:, i, lo - base : hi - base] = norm_a[:, lo:hi]
    meta = np.empty((NC, 128, 2 * M), dtype=np.float32)
    meta[:, :, 0::2] = md.transpose(0, 2, 1)
    meta[:, :, 1::2] = mn.transpose(0, 2, 1)

    Kb = np.array(
        [(off[(b + 1) * TILES] - off[b * TILES]) // 128 for b in range(NBLK1)]
    )
    assert Kb.sum() == C

    def wrap_idx(a):
        return np.ascontiguousarray(
            np.tile(a.reshape(NC, CAP // 16, 16).transpose(0, 2, 1), (1, 8, 1))
        )

    return dict(
        N=N, E=E, NSH=NSH, TILES=TILES, NBLK=NBLK, BLK=BLK, C=C, M=M,
        mm=mm, Kb=Kb, idx1_sb=wrap_idx(idx1_all), idx2_sb=wrap_idx(idx2_all),
        meta=meta, dinv=dinv.astype(np.float32), pos_all=pos_all,
    )


def _build_bass(sp, for_timing=False):
    import concourse.bass as bass
    import concourse.bacc as bacc
    import concourse.mybir as mybir
    import concourse.tile as tile

    f32 = mybir.dt.float32
    bf16 = mybir.dt.bfloat16
    i16 = mybir.dt.int16
    N, NSH, TILES = sp["N"], sp["NSH"], sp["TILES"]
    NBLK, BLK, C, M = sp["NBLK"], sp["BLK"], sp["C"], sp["M"]
    NBLK1 = NBLK + 1
    mm, Kb = sp["mm"], sp["Kb"]

    nc = bacc.Bacc("TRN2", target_bir_lowering=False, debug=False, num_devices=NC)
    x_in = nc.dram_tensor("x", [N, CIN], bf16, kind="ExternalInput")
    x_own_in = nc.dram_tensor("x_own", [NSH, CIN], bf16, kind="ExternalInput")
    idx1_in = nc.dram_tensor("idx1", [128, C * 8], i16, kind="ExternalInput")
    idx2_in = nc.dram_tensor("idx2", [128, C * 8], i16, kind="ExternalInput")
    meta_in = nc.dram_tensor("meta", [128, 2 * M], f32, kind="ExternalInput")
    wb_in = nc.dram_tensor("wb", [128, WBW], f32, kind="ExternalInput")
    out_d = nc.dram_tensor("out", [128, TILES * COUT], f32, kind="ExternalOutput")

    # per-block gather call lists (global chunk numbering, block-major)
    calls = []
    chunk0 = 0
    for b in range(NBLK1):
        kb = int(Kb[b])
        lst = []
        s0, left = chunk0, kb
        while left > 0:
            cs = min(CALL_CHUNKS, left)
            lst.append((s0, cs))
            s0 += cs
            left -= cs
        calls.append(lst)
        chunk0 += kb

    with tile.TileContext(nc) as tc:
        with (
            tc.tile_pool(name="const", bufs=1) as constp,
            tc.tile_pool(name="msgs", bufs=2) as msgsp,
            tc.tile_pool(name="work", bufs=20) as workp,
            tc.tile_pool(name="stage", bufs=3) as stagep,
            tc.tile_pool(name="pacc", bufs=4, space="PSUM") as paccp,
            tc.tile_pool(name="pproj", bufs=2, space="PSUM") as pprojp,
            tc.tile_pool(name="dram", bufs=1, space="DRAM") as dramp,
        ):
            wb = constp.tile([128, WBW], f32)
            nc.sync.dma_start(wb[:], wb_in[:])
            b1 = wb[:, 384:385]
            b2bc4 = wb[:, 486 : 486 + 4 * COUT]
            iota_bf = constp.tile([128, 128], bf16)
            nc.vector.tensor_copy(iota_bf[:], wb[:, 256:384])
            W1b = constp.tile([128, 128], bf16)
            nc.vector.tensor_copy(W1b[:], wb[:, 0:128])
            W2b = constp.tile([128, COUT], bf16)
            nc.vector.tensor_copy(W2b[:], wb[:, 128 : 128 + COUT])
            idxt1 = constp.tile([128, C * 8], i16)
            nc.sync.dma_start(idxt1[:], idx1_in[:])
            idxt2 = constp.tile([128, C * 8], i16)
            nc.sync.dma_start(idxt2[:], idx2_in[:])
            metab = constp.tile([128, 2 * M], f32)
            nc.sync.dma_start(metab[:], meta_in[:])

            g_shard = dramp.tile([NSH, 128], bf16)
            g_full = dramp.tile(
                [N, 128], bf16,
                addr_space="Local" if for_timing else "Shared",
            )

            def aggregate(tables, idxt, flip, epilogue):
                s_ci = [0] * NBLK1
                s_cur = [(-1, 0)] * NBLK1
                s_mg = [None] * NBLK1
                pt = None
                cur_t = -1
                for i in range(M):
                    b, t, c, lo, hi = (int(v) for v in mm[i])
                    cur0, curk = s_cur[b]
                    if s_mg[b] is None or c >= cur0 + curk:
                        cur0, curk = calls[b][s_ci[b]]
                        s_ci[b] += 1
                        s_cur[b] = (cur0, curk)
                        assert cur0 <= c < cur0 + curk
                        mg = msgsp.tile(
                            [128, CALL_CHUNKS, CIN], bf16, tag=f"msgs{b}"
                        )
                        s_mg[b] = mg
                        if b == 0:
                            src_ap = tables[0]
                        else:
                            base = (b - 1) * BLK
                            src_ap = tables[1][base : base + min(BLK, N - base), :]
                        nc.gpsimd.dma_gather(
                            mg[:, :curk, :],
                            src_ap,
                            idxt[:, cur0 * 8 : (cur0 + curk) * 8],
                            num_idxs=curk * 128,
                            num_idxs_reg=curk * 128,
                            elem_size=CIN,
                            single_packet=False,
                        )
                    if t != cur_t:
                        if pt is not None:
                            epilogue(cur_t, pt)
                        pt = paccp.tile([128, 128], f32, tag="pacc")
                        cur_t = t
                        first = True
                    else:
                        first = False
                    last = (i == M - 1) or (int(mm[i + 1][1]) != t)
                    cl = c - cur0
                    oh = workp.tile([128, 128], bf16, tag="oh")
                    eng = nc.gpsimd if (i % POOL_MOD) == 2 else nc.vector
                    eng.tensor_scalar(
                        oh[:],
                        iota_bf[:],
                        metab[:, 2 * i : 2 * i + 1],
                        metab[:, 2 * i + 1 : 2 * i + 2],
                        mybir.AluOpType.is_equal,
                        mybir.AluOpType.mult,
                    )
                    if not flip:
                        nc.tensor.matmul(
                            pt[:, :], s_mg[b][:, cl, :], oh[:],
                            start=first, stop=last,
                        )
                    else:
                        nc.tensor.matmul(
                            pt[:, :COUT], oh[:], s_mg[b][:, cl, :COUT],
                            start=first, stop=last,
                        )
                epilogue(cur_t, pt)

            # ---------------- layer 1 ----------------
            def epilogue1(t, pt):
                r0 = t * 128
                rows = min(128, NSH - r0)
                aggT = workp.tile([128, 128], bf16, tag="aggT")
                nc.scalar.activation(
                    aggT[:], pt[:], mybir.ActivationFunctionType.Copy
                )
                hp = pprojp.tile([128, 128], f32, tag="proj")
                nc.tensor.matmul(hp[:], W1b[:], aggT[:], start=True, stop=True)
                hs = workp.tile([128, 128], bf16, tag="hs")
                nc.scalar.activation(
                    hs[:], hp[:], mybir.ActivationFunctionType.Relu,
                    bias=b1, scale=1.0,
                )
                gp = pprojp.tile([128, COUT], f32, tag="projg")
                nc.tensor.matmul(gp[:], hs[:], W2b[:], start=True, stop=True)
                gs = workp.tile([128, 128], bf16, tag="gs")
                nc.scalar.activation(
                    gs[:, :COUT], gp[:], mybir.ActivationFunctionType.Copy,
                    scale=wb[:, 385 + t : 386 + t],
                )
                nc.vector.memset(gs[:, COUT:], 0)
                nc.sync.dma_start(g_shard[r0 : r0 + rows, :], gs[:rows, :])

            aggregate([x_own_in[:], x_in[:]], idxt1, False, epilogue1)

            # ---------------- exchange ----------------
            if for_timing:
                # dependency-only placeholder for the collective: touch one
                # row of every g_shard tile write so L2 gathers barrier on
                # all of layer 1, without charging DMA for a full local copy
                # (the real AllGather runs on the collective cores and is
                # accounted analytically by the harness).
                nc.gpsimd.dma_start(
                    g_full[0 : (TILES - 1) * 128 : 128, :],
                    g_shard[127 : (TILES - 1) * 128 : 128, :],
                )
                nc.gpsimd.dma_start(
                    g_full[NSH - 1 : NSH, :], g_shard[NSH - 1 : NSH, :]
                )
            else:
                nc.gpsimd.collective_compute(
                    "AllGather",
                    mybir.AluOpType.bypass,
                    replica_groups=[list(range(NC))],
                    ins=[g_shard[:]],
                    outs=[g_full[:]],
                )

            # ---------------- layer 2 ----------------
            ob_state = {}

            def epilogue2(t, pt):
                t0 = (t // 4) * 4
                if ob_state.get("t0") != t0:
                    obt = stagep.tile([128, 4 * COUT], f32, tag="ob")
                    ob_state["tile"] = obt
                    ob_state["t0"] = t0
                ot = ob_state["tile"]
                k = t - t0
                nc.scalar.activation(
                    ot[:, k * COUT : (k + 1) * COUT],
                    pt[:, :COUT],
                    mybir.ActivationFunctionType.Copy,
                )
                if t == TILES - 1 or k == 3:
                    nw = (k + 1) * COUT
                    nc.vector.tensor_tensor(
                        ot[:, :nw], ot[:, :nw], b2bc4[:, :nw],
                        mybir.AluOpType.add,
                    )
                    nc.sync.dma_start(
                        out_d[:, t0 * COUT : t0 * COUT + nw], ot[:, :nw]
                    )

            aggregate([g_shard[:], g_full[:]], idxt2, True, epilogue2)

    nc.compile()
    return nc


_CACHE = {}


def _get_program(sp):
    key = (sp["N"], sp["C"], sp["mm"].tobytes())
    if key not in _CACHE:
        _CACHE[key] = _build_bass(sp)
    return _CACHE[key]


def _make_wb(sp, W1, b1, W2, b2):
    NSH, TILES = sp["NSH"], sp["TILES"]
    wb = np.zeros((NC, 128, WBW), dtype=np.float32)
    wb[:, :, 0:128] = np.asarray(W1, dtype=np.float32)[None]
    wb[:, :, 128 : 128 + COUT] = np.asarray(W2, dtype=np.float32)[None]
    wb[:, :, 256:384] = np.arange(128, dtype=np.float32)[None, None, :]
    wb[:, :, 384] = np.asarray(b1, dtype=np.float32)[None]
    dinv = sp["dinv"]
    pos_all = sp["pos_all"]
    dv = np.zeros((NC, TILES * 128), dtype=np.float32)
    for r in range(NC):
        dv[r, pos_all[r]] = dinv[r * NSH : (r + 1) * NSH]
    wb[:, :, 385 : 385 + TILES] = dv.reshape(NC, TILES, 128).transpose(0, 2, 1)
    wb[:, :, 486 : 486 + 4 * COUT] = np.tile(
        np.asarray(b2, dtype=np.float32), 4
    )[None, None, :]
    return wb


def make_in_maps(sp, x, W1, b1, W2, b2):
    dinv = sp["dinv"]
    NSH = sp["NSH"]
    xs = (np.asarray(x, dtype=np.float32) * dinv[:, None]).astype(BF16)
    xs = np.ascontiguousarray(xs)
    wb = _make_wb(sp, W1, b1, W2, b2)
    return [
        {
            "x": xs,
            "x_own": np.ascontiguousarray(xs[r * NSH : (r + 1) * NSH]),
            "idx1": sp["idx1_sb"][r],
            "idx2": sp["idx2_sb"][r],
            "meta": sp["meta"][r],
            "wb": wb[r],
        }
        for r in range(NC)
    ]


def kernel(x, edge_index, W1, b1, W2, b2, _trace=False):
    from concourse.bass_utils import run_bass_kernel_spmd

    x = np.asarray(x, dtype=np.float32)
    N = x.shape[0]
    sp = _schedule(np.asarray(edge_index), N)
    nc = _get_program(sp)
    in_maps = make_in_maps(sp, x, W1, b1, W2, b2)
    res = run_bass_kernel_spmd(nc, in_maps, list(range(NC)), trace=_trace)

    NSH, TILES = sp["NSH"], sp["TILES"]
    out = np.empty((N, COUT), dtype=np.float32)
    for r in range(NC):
        o = res.results[r]["out"].reshape(128, TILES, COUT)
        full = o.transpose(1, 0, 2).reshape(TILES * 128, COUT)
        out[r * NSH : (r + 1) * NSH] = full[sp["pos_all"][r]]
    if _trace:
        kernel.last_result = res
    return out


# revision 39
# speedup vs baseline: 1.0089x; 1.0089x over previous
"""2-layer GCN on 8 Trainium2 NeuronCores (Bass/Tile).

Math: gcn_conv(x, W, b) = D^-1/2 (A+I) D^-1/2 (x W) + b.  With
x~ = dinv*x and g~ = dinv*(h W2)  (both tables dinv-row-scaled):
  h[v]   = relu((sum_{u->v} dinv_v x~[u] + dinv_v x~[v]) W1 + b1)
  out[v] = sum_{u->v} dinv_v g~[u] + dinv_v g~[v] + b2
Self-loops ride the edge streams as ordinary edges; dinv_v lives in the
one-hot values, dinv_u in the gathered tables.

Distribution: dst nodes sharded 8 ways (12500/core, identity layout).
g~ is stored 128-wide (64 real cols) so rows are 256B for dma_gather;
one 8-rank AllGather (25.6MB) assembles g_full between the layers.

ONE shared edge schedule serves both layers (same src row index per
edge, tables have identical [N,128] shape): per-core slot order, idx
(int16 into 4 row-blocks of 25000) and meta ([dstl, dinv_dst] bf16) are
built once on the host and PRELOADED to SBUF (no per-call idx/meta
streams).  Consumption is tile-major: per mm entry a one-hot
(is_equal x dinv_dst, bf16) routes 128 gathered rows to their dst
columns via a PE matmul accumulating in PSUM.
- L1: stationary = msgs, moving = one-hot -> pt [feat, dst];
  epilogue: W1 -> relu -> W2 -> g~ rows (ACT per-partition dinv scale).
- L2: FLIPPED: stationary = one-hot, moving = msgs[:, :64] ->
  pt [dst, 64]; epilogue: +b2 (one DVE add per 4 tiles), staged output
  flushes of 1KB/partition.
- One-hot builds alternate DVE / Pool (GPSIMD) so neither sequencer
  bottlenecks behind the gather DMA stream.
"""
import sys

sys.path.insert(0, "/opt/trn_rl_repo")
import numpy as np
import ml_dtypes

BF16 = ml_dtypes.bfloat16
NC = 8
CIN, CH, COUT = 128, 128, 64
CALL_CHUNKS = 40  # gather-call granularity (chunks of 128 descs)
POOL_MOD = 3  # mm entry i builds on Pool when i % POOL_MOD == 2
WBW = 752  # wb cols: W1 0:128 | W2 128:192 | iota 256:384 | b1 384 |
#            b2bc4 486:742


def _balance(nv, caps, k_tb):
    """Best-fit-decreasing node->tile assignment for one core.

    nv: [NSH, NBLK] per-node block in-degree vectors.
    caps: [TILES] node slots per tile.  k_tb: [TILES, NBLK] edge caps.
    Returns pos[NSH] (node -> slot = tile*128 + slot_in_tile) or None.
    """
    NSH, NBLK = nv.shape
    TILES = caps.shape[0]
    capv = k_tb.astype(np.float64)
    np.maximum(capv, 1e-9, out=capv)
    usedv = np.zeros((TILES, NBLK), dtype=np.float64)
    usect = np.zeros(TILES, dtype=np.int64)
    pos = np.empty(NSH, dtype=np.int64)
    order = np.argsort(-nv.sum(1), kind="stable")
    for n in order:
        v = nv[n].astype(np.float64)
        post = (usedv + v[None, :]) / capv
        feas = (post <= 1.0).all(1) & (usect < caps)
        if not feas.any():
            return None
        score = np.maximum(post.max(1), (usect + 1) / caps)
        score[~feas] = np.inf
        t = int(np.argmin(score))
        usedv[t] += v
        pos[n] = t * 128 + usect[t]
        usect[t] += 1
    return pos


def _schedule(edge_index, n_nodes):
    N = n_nodes
    NSH = N // NC
    TILES = (NSH + 127) // 128
    assert N % NC == 0

    src = np.asarray(edge_index[0], dtype=np.int64)
    dst = np.asarray(edge_index[1], dtype=np.int64)
    E = src.shape[0]
    deg = np.bincount(dst, minlength=N).astype(np.float64) + 1.0
    dinv = 1.0 / np.sqrt(deg)

    s = src
    d = dst
    EA = s.shape[0]

    core = d // NSH
    j = d - core * NSH
    norm = dinv[d].astype(np.float32)

    NBLK = 4
    BLK = (N + NBLK - 1) // NBLK
    assert BLK <= 32767 and (NSH % BLK == 0 or BLK % NSH == 0)
    m1 = s % BLK
    blk = s // BLK  # src block 0..3; self-loops get dedicated block below
    ncell = NBLK * TILES

    # node -> tile permutation balancing per-(tile, block) cell loads
    nv = np.bincount(
        (core * NSH + j) * NBLK + blk, minlength=NC * NSH * NBLK
    ).reshape(NC, NSH, NBLK)
    B = nv.sum(axis=1)  # [NC, NBLK]
    caps_n = np.full(TILES, 128, dtype=np.int64)
    caps_n[TILES - 1] = NSH - 128 * (TILES - 1)
    slack = 512
    for _attempt in range(8):
        TOTb = ((B.max(axis=0) + slack + 127) // 128) * 128
        captb = np.zeros((NBLK, TILES), dtype=np.int64)
        for b in range(NBLK):
            tgt = TOTb[b] * caps_n / NSH
            base = np.floor(tgt).astype(np.int64)
            rem = int(TOTb[b] - base.sum())
            order_t = np.argsort(-(tgt - base))
            base[order_t[:rem]] += 1
            captb[b] = base
        pos_all = np.empty((NC, NSH), dtype=np.int64)
        ok = True
        for r in range(NC):
            pos = _balance(nv[r], caps_n, captb.T)
            if pos is None:
                ok = False
                break
            pos_all[r] = pos
        if ok:
            break
        slack *= 2
    assert ok, "balance failed"

    pos_d = pos_all[core, j]
    t_dst = pos_d >> 7
    dstl = (pos_d & 127).astype(np.float32)
    # block 0 = SELF (tile-aligned, 128/tile); blocks 1..4 = src ranges
    cap_self = np.full((1, TILES), 128, dtype=np.int64)
    captb5 = np.concatenate([cap_self, captb], axis=0)  # [NBLK+1, TILES]
    NBLK1 = NBLK + 1
    ncell1 = NBLK1 * TILES
    cell = (blk + 1) * TILES + t_dst
    caps = captb5.reshape(-1)  # [ncell1], block-major

    off = np.zeros(ncell1 + 1, dtype=np.int64)
    np.cumsum(caps, out=off[1:])
    CAP = int(off[-1])
    assert CAP % 128 == 0
    C = CAP // 128

    # slot: per (core, cell), streamed edges laid in (m1) order
    key = (core * ncell1 + cell) * BLK + m1
    order = np.argsort(key, kind="stable")
    ckey = (core * ncell1 + cell)[order]
    cell_first = np.zeros(EA, dtype=bool)
    cell_first[0] = True
    np.not_equal(ckey[1:], ckey[:-1], out=cell_first[1:])
    cstart = np.nonzero(cell_first)[0]
    c_id = np.cumsum(cell_first) - 1
    pos_in_cell = np.arange(EA, dtype=np.int64) - cstart[c_id]
    assert (pos_in_cell < caps[ckey % ncell1]).all()
    slot_sorted = off[ckey % ncell1] + pos_in_cell
    slot = np.empty(EA, dtype=np.int64)
    slot[order] = slot_sorted

    # L2 source row: permuted shard-major g_full row, same block as src
    sc = s // NSH
    sj = s - sc * NSH
    g_row = sc * NSH + pos_all[sc, sj]
    m2 = g_row - blk * BLK
    assert (m2 >= 0).all() and (m2 < BLK).all()

    idx1_all = np.zeros((NC, CAP), dtype=np.int16)
    idx2_all = np.zeros((NC, CAP), dtype=np.int16)
    dstl_a = np.full((NC, CAP), -1.0, dtype=np.float32)
    norm_a = np.zeros((NC, CAP), dtype=np.float32)
    idx1_all[core, slot] = m1.astype(np.int16)
    idx2_all[core, slot] = m2.astype(np.int16)
    dstl_a[core, slot] = dstl
    norm_a[core, slot] = norm
    # self block: node with permuted position p sits at slot p; idx1 = its
    # original j (row of x_own), idx2 = p (row of g_shard), value = dinv
    for r in range(NC):
        jj = np.arange(NSH, dtype=np.int64)
        p = pos_all[r]
        idx1_all[r, p] = jj.astype(np.int16)
        idx2_all[r, p] = p.astype(np.int16)
        dstl_a[r, p] = (p & 127).astype(np.float32)
        norm_a[r, p] = dinv[r * NSH + jj].astype(np.float32)

    # mm entries, tile-major consumption order
    mm = []
    for t in range(TILES):
        for b in range(NBLK1):
            o = int(off[b * TILES + t])
            cap = int(caps[b * TILES + t])
            if cap == 0:
                continue
            c0, c1 = o >> 7, (o + cap - 1) >> 7
            for c in range(c0, c1 + 1):
                lo = max(o, c * 128)
                hi = min(o + cap, (c + 1) * 128)
                mm.append((b, t, c, lo, hi))
    mm = np.array(mm, dtype=np.int64)
    M = mm.shape[0]

    md = np.full((NC, M, 128), -1.0, dtype=np.float32)
    mn = np.zeros((NC, M, 128), dtype=np.float32)
    for i in range(M):
        b, t, c, lo, hi = mm[i]
        base = int(c) * 128
        md[:, i, lo - base : hi - base] = dstl_a[:, lo:hi]
        mn[:, i, lo - base : hi - base] = norm_a[:, lo:hi]
    meta = np.empty((NC, 128, 2 * M), dtype=np.float32)
    meta[:, :, 0::2] = md.transpose(0, 2, 1)
    meta[:, :, 1::2] = mn.transpose(0, 2, 1)

    Kb = np.array(
        [(off[(b + 1) * TILES] - off[b * TILES]) // 128 for b in range(NBLK1)]
    )
    assert Kb.sum() == C

    def wrap_idx(a):
        return np.ascontiguousarray(
            np.tile(a.reshape(NC, CAP // 16, 16).transpose(0, 2, 1), (1, 8, 1))
        )

    return dict(
        N=N, E=E, NSH=NSH, TILES=TILES, NBLK=NBLK, BLK=BLK, C=C, M=M,
        mm=mm, Kb=Kb, idx1_sb=wrap_idx(idx1_all), idx2_sb=wrap_idx(idx2_all),
        meta=meta, dinv=dinv.astype(np.float32), pos_all=pos_all,
    )


def _build_bass(sp, for_timing=False):
    import concourse.bass as bass
    import concourse.bacc as bacc
    import concourse.mybir as mybir
    import concourse.tile as tile

    f32 = mybir.dt.float32
    bf16 = mybir.dt.bfloat16
    i16 = mybir.dt.int16
    N, NSH, TILES = sp["N"], sp["NSH"], sp["TILES"]
    NBLK, BLK, C, M = sp["NBLK"], sp["BLK"], sp["C"], sp["M"]
    NBLK1 = NBLK + 1
    mm, Kb = sp["mm"], sp["Kb"]

    nc = bacc.Bacc("TRN2", target_bir_lowering=False, debug=False, num_devices=NC)
    x_in = nc.dram_tensor("x", [N, CIN], bf16, kind="ExternalInput")
    x_own_in = nc.dram_tensor(
        "x_own", [128, TILES * 128], bf16, kind="ExternalInput"
    )
    idx1_in = nc.dram_tensor("idx1", [128, C * 8], i16, kind="ExternalInput")
    idx2_in = nc.dram_tensor("idx2", [128, C * 8], i16, kind="ExternalInput")
    meta_in = nc.dram_tensor("meta", [128, 2 * M], f32, kind="ExternalInput")
    wb_in = nc.dram_tensor("wb", [128, WBW], f32, kind="ExternalInput")
    out_d = nc.dram_tensor("out", [128, TILES * COUT], f32, kind="ExternalOutput")

    # per-block gather call lists (global chunk numbering, block-major)
    calls = []
    chunk0 = 0
    for b in range(NBLK1):
        kb = int(Kb[b])
        lst = []
        s0, left = chunk0, kb
        while left > 0:
            cs = min(CALL_CHUNKS, left)
            lst.append((s0, cs))
            s0 += cs
            left -= cs
        calls.append(lst)
        chunk0 += kb

    with tile.TileContext(nc) as tc:
        with (
            tc.tile_pool(name="const", bufs=1) as constp,
            tc.tile_pool(name="msgs", bufs=2) as msgsp,
            tc.tile_pool(name="work", bufs=20) as workp,
            tc.tile_pool(name="stage", bufs=3) as stagep,
            tc.tile_pool(name="pacc", bufs=4, space="PSUM") as paccp,
            tc.tile_pool(name="pproj", bufs=2, space="PSUM") as pprojp,
            tc.tile_pool(name="dram", bufs=1, space="DRAM") as dramp,
        ):
            wb = constp.tile([128, WBW], f32)
            nc.sync.dma_start(wb[:], wb_in[:])
            b1 = wb[:, 384:385]
            b2bc4 = wb[:, 486 : 486 + 4 * COUT]
            iota_bf = constp.tile([128, 128], bf16)
            nc.vector.tensor_copy(iota_bf[:], wb[:, 256:384])
            W1b = constp.tile([128, 128], bf16)
            nc.vector.tensor_copy(W1b[:], wb[:, 0:128])
            W2b = constp.tile([128, COUT], bf16)
            nc.vector.tensor_copy(W2b[:], wb[:, 128 : 128 + COUT])
            idxt1 = constp.tile([128, C * 8], i16)
            nc.sync.dma_start(idxt1[:], idx1_in[:])
            idxt2 = constp.tile([128, C * 8], i16)
            nc.sync.dma_start(idxt2[:], idx2_in[:])
            metab = constp.tile([128, 2 * M], f32)
            nc.sync.dma_start(metab[:], meta_in[:])

            g_shard = dramp.tile([NSH, 128], bf16)
            g_full = dramp.tile(
                [N, 128], bf16,
                addr_space="Local" if for_timing else "Shared",
            )

            def aggregate(tables, idxt, flip, epilogue, self_copy=False):
                s_ci = [0] * NBLK1
                s_cur = [(-1, 0)] * NBLK1
                s_mg = [None] * NBLK1
                pt = None
                cur_t = -1
                for i in range(M):
                    b, t, c, lo, hi = (int(v) for v in mm[i])
                    cur0, curk = s_cur[b]
                    if s_mg[b] is None or c >= cur0 + curk:
                        cur0, curk = calls[b][s_ci[b]]
                        s_ci[b] += 1
                        s_cur[b] = (cur0, curk)
                        assert cur0 <= c < cur0 + curk
                        mg = msgsp.tile(
                            [128, CALL_CHUNKS, CIN], bf16, tag=f"msgs{b}"
                        )
                        s_mg[b] = mg
                        if b == 0 and self_copy:
                            # self rows are tile-aligned in permuted order:
                            # chunk == tile, so the whole call is one
                            # contiguous large-element copy
                            nc.sync.dma_start(
                                mg[:, :curk, :],
                                tables[0][
                                    :, cur0 * 128 : (cur0 + curk) * 128
                                ].rearrange("p (k f) -> p k f", f=CIN),
                            )
                        else:
                            if b == 0:
                                src_ap = tables[0]
                            else:
                                base = (b - 1) * BLK
                                src_ap = tables[1][
                                    base : base + min(BLK, N - base), :
                                ]
                            nc.gpsimd.dma_gather(
                                mg[:, :curk, :],
                                src_ap,
                                idxt[:, cur0 * 8 : (cur0 + curk) * 8],
                                num_idxs=curk * 128,
                                num_idxs_reg=curk * 128,
                                elem_size=CIN,
                                single_packet=False,
                            )
                    if t != cur_t:
                        if pt is not None:
                            epilogue(cur_t, pt)
                        pt = paccp.tile([128, 128], f32, tag="pacc")
                        cur_t = t
                        first = True
                    else:
                        first = False
                    last = (i == M - 1) or (int(mm[i + 1][1]) != t)
                    cl = c - cur0
                    oh = workp.tile([128, 128], bf16, tag="oh")
                    eng = nc.gpsimd if (i % POOL_MOD) == 2 else nc.vector
                    eng.tensor_scalar(
                        oh[:],
                        iota_bf[:],
                        metab[:, 2 * i : 2 * i + 1],
                        metab[:, 2 * i + 1 : 2 * i + 2],
                        mybir.AluOpType.is_equal,
                        mybir.AluOpType.mult,
                    )
                    if not flip:
                        nc.tensor.matmul(
                            pt[:, :], s_mg[b][:, cl, :], oh[:],
                            start=first, stop=last,
                        )
                    else:
                        nc.tensor.matmul(
                            pt[:, :COUT], oh[:], s_mg[b][:, cl, :COUT],
                            start=first, stop=last,
                        )
                epilogue(cur_t, pt)

            # ---------------- layer 1 ----------------
            def epilogue1(t, pt):
                r0 = t * 128
                rows = min(128, NSH - r0)
                aggT = workp.tile([128, 128], bf16, tag="aggT")
                nc.scalar.activation(
                    aggT[:], pt[:], mybir.ActivationFunctionType.Copy
                )
                hp = pprojp.tile([128, 128], f32, tag="proj")
                nc.tensor.matmul(hp[:], W1b[:], aggT[:], start=True, stop=True)
                hs = workp.tile([128, 128], bf16, tag="hs")
                nc.scalar.activation(
                    hs[:], hp[:], mybir.ActivationFunctionType.Relu,
                    bias=b1, scale=1.0,
                )
                gp = pprojp.tile([128, COUT], f32, tag="projg")
                nc.tensor.matmul(gp[:], hs[:], W2b[:], start=True, stop=True)
                gs = workp.tile([128, 128], bf16, tag="gs")
                nc.scalar.activation(
                    gs[:, :COUT], gp[:], mybir.ActivationFunctionType.Copy,
                    scale=wb[:, 385 + t : 386 + t],
                )
                nc.vector.memset(gs[:, COUT:], 0)
                nc.sync.dma_start(g_shard[r0 : r0 + rows, :], gs[:rows, :])

            aggregate(
                [x_own_in[:], x_in[:]], idxt1, False, epilogue1, self_copy=True
            )

            # ---------------- exchange ----------------
            if for_timing:
                # dependency-only placeholder for the collective: touch one
                # row of every g_shard tile write so L2 gathers barrier on
                # all of layer 1, without charging DMA for a full local copy
                # (the real AllGather runs on the collective cores and is
                # accounted analytically by the harness).
                nc.gpsimd.dma_start(
                    g_full[0 : (TILES - 1) * 128 : 128, :],
                    g_shard[127 : (TILES - 1) * 128 : 128, :],
                )
                nc.gpsimd.dma_start(
                    g_full[NSH - 1 : NSH, :], g_shard[NSH - 1 : NSH, :]
                )
            else:
                nc.gpsimd.collective_compute(
                    "AllGather",
                    mybir.AluOpType.bypass,
                    replica_groups=[list(range(NC))],
                    ins=[g_shard[:]],
                    outs=[g_full[:]],
                )

            # ---------------- layer 2 ----------------
            ob_state = {}

            def epilogue2(t, pt):
                t0 = (t // 4) * 4
                if ob_state.get("t0") != t0:
                    obt = stagep.tile([128, 4 * COUT], f32, tag="ob")
                    ob_state["tile"] = obt
                    ob_state["t0"] = t0
                ot = ob_state["tile"]
                k = t - t0
                nc.scalar.activation(
                    ot[:, k * COUT : (k + 1) * COUT],
                    pt[:, :COUT],
                    mybir.ActivationFunctionType.Copy,
                )
                if t == TILES - 1 or k == 3:
                    nw = (k + 1) * COUT
                    nc.vector.tensor_tensor(
                        ot[:, :nw], ot[:, :nw], b2bc4[:, :nw],
                        mybir.AluOpType.add,
                    )
                    nc.sync.dma_start(
                        out_d[:, t0 * COUT : t0 * COUT + nw], ot[:, :nw]
                    )

            aggregate([g_shard[:], g_full[:]], idxt2, True, epilogue2)

    nc.compile()
    return nc


_CACHE = {}


def _get_program(sp):
    key = (sp["N"], sp["C"], sp["mm"].tobytes())
    if key not in _CACHE:
        _CACHE[key] = _build_bass(sp)
    return _CACHE[key]


def _make_wb(sp, W1, b1, W2, b2):
    NSH, TILES = sp["NSH"], sp["TILES"]
    wb = np.zeros((NC, 128, WBW), dtype=np.float32)
    wb[:, :, 0:128] = np.asarray(W1, dtype=np.float32)[None]
    wb[:, :, 128 : 128 + COUT] = np.asarray(W2, dtype=np.float32)[None]
    wb[:, :, 256:384] = np.arange(128, dtype=np.float32)[None, None, :]
    wb[:, :, 384] = np.asarray(b1, dtype=np.float32)[None]
    dinv = sp["dinv"]
    pos_all = sp["pos_all"]
    dv = np.zeros((NC, TILES * 128), dtype=np.float32)
    for r in range(NC):
        dv[r, pos_all[r]] = dinv[r * NSH : (r + 1) * NSH]
    wb[:, :, 385 : 385 + TILES] = dv.reshape(NC, TILES, 128).transpose(0, 2, 1)
    wb[:, :, 486 : 486 + 4 * COUT] = np.tile(
        np.asarray(b2, dtype=np.float32), 4
    )[None, None, :]
    return wb


def _x_own_perm(sp, xs, r):
    """Self-block table: row (p, t*128+f) = x~ of the node at permuted
    position t*128+p (zeros on dead slots)."""
    NSH, TILES = sp["NSH"], sp["TILES"]
    arr = np.zeros((TILES * 128, CIN), dtype=BF16)
    arr[sp["pos_all"][r]] = xs[r * NSH : (r + 1) * NSH]
    return np.ascontiguousarray(
        arr.reshape(TILES, 128, CIN).transpose(1, 0, 2).reshape(128, TILES * CIN)
    )


def make_in_maps(sp, x, W1, b1, W2, b2):
    dinv = sp["dinv"]
    NSH = sp["NSH"]
    xs = (np.asarray(x, dtype=np.float32) * dinv[:, None]).astype(BF16)
    xs = np.ascontiguousarray(xs)
    wb = _make_wb(sp, W1, b1, W2, b2)
    return [
        {
            "x": xs,
            "x_own": _x_own_perm(sp, xs, r),
            "idx1": sp["idx1_sb"][r],
            "idx2": sp["idx2_sb"][r],
            "meta": sp["meta"][r],
            "wb": wb[r],
        }
        for r in range(NC)
    ]


def kernel(x, edge_index, W1, b1, W2, b2, _trace=False):
    from concourse.bass_utils import run_bass_kernel_spmd

    x = np.asarray(x, dtype=np.float32)
    N = x.shape[0]
    sp = _schedule(np.asarray(edge_index), N)
    nc = _get_program(sp)
    in_maps = make_in_maps(sp, x, W1, b1, W2, b2)
    res = run_bass_kernel_spmd(nc, in_maps, list(range(NC)), trace=_trace)

    NSH, TILES = sp["NSH"], sp["TILES"]
    out = np.empty((N, COUT), dtype=np.float32)
    for r in range(NC):
        o = res.results[r]["out"].reshape(128, TILES, COUT)
        full = o.transpose(1, 0, 2).reshape(TILES * 128, COUT)
        out[r * NSH : (r + 1) * NSH] = full[sp["pos_all"][r]]
    if _trace:
        kernel.last_result = res
    return out
